# revision 1
# baseline (speedup 1.0000x reference)
"""Evoformer block distributed across 8 trn2 NeuronCores.

Sharding (DAP-style per FastFold):
  - MSA ops: m [N_seq=128, N_res=256, C_M=256] sharded over N_seq (16 rows/core)
    for row attention / transition; column attention + outer-product-mean
    contract over N_seq, handled via resharding (all-to-all) to an N_res shard.
  - Pair stack: z [256, 256, 128] sharded over the first residue axis
    (32 rows/core); triangle mult/attention contract over the full residue
    axis -> all-gathers around them.
The graph is expressed with sharding constraints on a 1-D mesh of the 8
cores; XLA GSPMD inserts the all-gather/all-to-all collectives.
"""
import numpy as np
import jax
import jax.numpy as jnp
from jax.sharding import Mesh, NamedSharding, PartitionSpec as P

C_M, C_Z = 256, 128
N_SEQ, N_RES = 128, 256
NCORES = 8

# ---------------- forward math (mirrors the reference) ----------------

def _ln(x, p, eps=1e-5):
    mu = jnp.mean(x, -1, keepdims=True)
    var = jnp.mean(jnp.square(x - mu), -1, keepdims=True)
    return (x - mu) * jax.lax.rsqrt(var + eps) * p["g"] + p["b"]


def _lin(x, p):
    y = x @ p["w"]
    return y + p["b"] if "b" in p else y


def _mha(x, p, attn_dim, n_head, c, bias=None):
    x = jnp.moveaxis(x, attn_dim, -2)

    def split_heads(t):
        t = t.reshape(t.shape[:-1] + (n_head, c))
        return jnp.moveaxis(t, -2, -3)

    q = split_heads(_lin(x, p["q"]))
    k = split_heads(_lin(x, p["k"]))
    v = split_heads(_lin(x, p["v"]))
    s = jnp.einsum("...hqc,...hkc->...hqk", q, k) / np.float32(np.sqrt(c))
    if bias is not None:
        s = s + bias
    a = jax.nn.softmax(s, axis=-1)
    o = jnp.einsum("...hqk,...hkc->...hqc", a, v)
    o = jnp.moveaxis(o, -3, -2)
    o = o.reshape(o.shape[:-2] + (n_head * c,))
    o = jax.nn.sigmoid(_lin(x, p["g"])) * o
    out = _lin(o, p["o"])
    return jnp.moveaxis(out, -2, attn_dim)


def _transition(x, p):
    x = _ln(x, p["ln"])
    return _lin(jax.nn.relu(_lin(x, p["l1"])), p["l2"])


def _opm(m, p):
    n_seq = m.shape[-3]
    mn = _ln(m, p["ln"])
    a = _lin(mn, p["l1"])
    b = _lin(mn, p["l2"])
    o = jnp.einsum("...sic,...sjd->...ijcd", a, b)
    o = o.reshape(o.shape[:-2] + (a.shape[-1] * b.shape[-1],))
    return _lin(o, p["out"]) / n_seq


def _tri_mul(z, p, outgoing):
    zn = _ln(z, p["ln_in"])
    a = jax.nn.sigmoid(_lin(zn, p["ag"])) * _lin(zn, p["ap"])
    b = jax.nn.sigmoid(_lin(zn, p["bg"])) * _lin(zn, p["bp"])
    g = jax.nn.sigmoid(_lin(zn, p["g"]))
    if outgoing:
        t = jnp.einsum("...ikc,...jkc->...ijc", a, b)
    else:
        t = jnp.einsum("...kic,...kjc->...ijc", a, b)
    return g * _lin(_ln(t, p["ln_out"]), p["out"])


def _tri_att(z, p, starting):
    zn = _ln(z, p["ln"])
    bias = jnp.moveaxis(_lin(zn, p["lin_b"]), -1, -3)
    if starting:
        return _mha(zn, p["mha"], -2, 4, 32, bias=bias)
    return _mha(zn, p["mha"], -3, 4, 32, bias=jnp.swapaxes(bias, -1, -2))


def _forward(m, z, params):
    shard_s = lambda t: jax.lax.with_sharding_constraint(
        t, NamedSharding(_MESH, P("x", None, None)))
    shard_i = lambda t: jax.lax.with_sharding_constraint(
        t, NamedSharding(_MESH, P(None, "x", None)))

    # MSA row attention with pair bias (rows independent -> shard N_seq)
    Pp = params["row"]
    m = shard_s(m)
    mn = _ln(m, Pp["ln_m"])
    b = jnp.moveaxis(_lin(_ln(z, Pp["ln_z"]), Pp["lin_z"]), -1, -3)
    m = m + _mha(mn, Pp["mha"], -2, 8, 32, bias=b)
    m = shard_s(m)
    # MSA column attention (columns independent -> shard N_res)
    Pp = params["col"]
    m = shard_i(m)
    m = m + _mha(_ln(m, Pp["ln"]), Pp["mha"], -3, 8, 32)
    m = shard_i(m)
    # MSA transition (elementwise over tokens)
    m = m + _transition(m, params["msa_trans"])
    # outer product mean -> pair update (contract N_seq; output row-sharded)
    z = z + _opm(m, params["opm"])
    z = jax.lax.with_sharding_constraint(
        z, NamedSharding(_MESH, P("x", None, None)))
    # pair stack: z row-sharded; all-gathers inserted where j-axis is needed
    z = z + _tri_mul(z, params["tmo"], True)
    z = z + _tri_mul(z, params["tmi"], False)
    z = z + _tri_att(z, params["tas"], True)
    z = z + _tri_att(z, params["tae"], False)
    z = z + _transition(z, params["pair_trans"])
    z = jax.lax.with_sharding_constraint(
        z, NamedSharding(_MESH, P("x", None, None)))
    return (m, z)


_MESH = None
_JITTED = None


def _get_jitted():
    global _MESH, _JITTED
    if _JITTED is not None:
        return _JITTED
    devs = jax.devices()[:NCORES]
    _MESH = Mesh(np.asarray(devs), ("x",))
    repl = NamedSharding(_MESH, P())
    m_shard = NamedSharding(_MESH, P("x", None, None))
    z_shard = NamedSharding(_MESH, P("x", None, None))
    _JITTED = jax.jit(
        _forward,
        in_shardings=(m_shard, z_shard, repl),
        out_shardings=(NamedSharding(_MESH, P(None, "x", None)), z_shard),
    )
    return _JITTED


def kernel(m, z, params):
    fn = _get_jitted()
    m = jnp.asarray(np.asarray(m), jnp.float32)
    z = jnp.asarray(np.asarray(z), jnp.float32)
    params = jax.tree.map(lambda t: jnp.asarray(np.asarray(t), jnp.float32), params)
    mo, zo = fn(m, z, params)
    return (np.asarray(jax.device_get(mo)), np.asarray(jax.device_get(zo)))
